# revision 1
# baseline (speedup 1.0000x reference)
"""ArcFace logits kernel for 8 trn2 NeuronCores (class-axis model parallel).

kernel(input, weight, label) -> [1024, 100000] f32 scaled-margin logits.

Device work per core (SPMD over 8 cores):
  - x [1024,512] f32 (replicated): row-l2-normalize (x30 scale folded in),
    cast bf16, PE-transpose -> xnT [512,1024] bf16.
  - w shard [12544,512] f32 (12500 real rows + zero pad): stream in 1MB
    tiles, row-l2-normalize, cast bf16, PE-transpose -> wT [512, c] bf16.
  - cosine slab out[n, c] = xnT.T @ wT accumulated f32 in PSUM over 4
    k-chunks, evicted to SBUF (ACT/DVE), DMA'd to DRAM ([1024, 12500]).
Host: concatenate 8 slabs, then apply the ArcFace margin at the 1024
labeled positions using the device-computed cosines (same math as ref).

Engine budget per W chunk (512 classes): PE 32 MM + 16 transposes;
DVE sumsq+recip+wT-evict+2 out-evicts; ACT sqrt+6 out-evicts;
GPSIMD W-load DMA (SWDGE ring, separate from out-write HWDGE FIFO)
+ normalize-mult; SP HWDGE out writes.
"""

import math
from contextlib import ExitStack

import numpy as np

import concourse.bass as bass
import concourse.bacc as bacc
import concourse.mybir as mybir
from concourse.masks import make_identity
from concourse.tile import TileContext
from concourse.bass_utils import run_bass_kernel_spmd

F32 = mybir.dt.float32
BF16 = mybir.dt.bfloat16

N = 1024          # batch
D = 512           # in_features
C_TOTAL = 100000  # out_features
N_CORES = 8
C_PER = C_TOTAL // N_CORES     # 12500 real classes per core
C_PAD = 12800                  # 25 chunks of 512 (uniform; partial-width
                               # chunks hang the HW DMA path — see notes)
KT = D // 128                  # 4 k-chunks

SCALE = 30.0
MARGIN = 0.5
COS_M = math.cos(MARGIN)
SIN_M = math.sin(MARGIN)
TH = math.cos(math.pi - MARGIN)
MM = math.sin(math.pi - MARGIN) * MARGIN


def build_nc(n=N, d=D, c_pad=C_PAD, c_out=C_PER, swdge_w=True, use_ttr=False):
    # NOTE: use_ttr=True (nc.vector.tensor_tensor_reduce) passes CoreSim but
    # wedges real HW (NRT_EXEC_UNIT_UNRECOVERABLE) — do not enable.
    # Partial-width W chunks (c_pad not a multiple of 512) also hang HW.
    nt = n // 128
    kt = d // 128
    n_chunks = (c_pad + 511) // 512

    nc = bacc.Bacc(None, target_bir_lowering=False, debug=False)
    x = nc.declare_dram_parameter("x", [n, d], F32, isOutput=False)
    w = nc.declare_dram_parameter("w", [c_pad, d], F32, isOutput=False)
    out = nc.declare_dram_parameter("out", [n, c_out], F32, isOutput=True)

    with ExitStack() as ctx:
        tc = ctx.enter_context(TileContext(nc))

        consts = ctx.enter_context(tc.tile_pool(name="consts", bufs=1))
        xpool = ctx.enter_context(tc.tile_pool(name="xpool", bufs=3))
        stats = ctx.enter_context(tc.tile_pool(name="stats", bufs=16))
        xnt_pool = ctx.enter_context(tc.tile_pool(name="xnt", bufs=1))
        wpool = ctx.enter_context(tc.tile_pool(name="wpool", bufs=6))
        wbf_pool = ctx.enter_context(tc.tile_pool(name="wbf", bufs=6))
        wt2_pool = ctx.enter_context(tc.tile_pool(name="wt2", bufs=4))
        opool = ctx.enter_context(tc.tile_pool(name="opool", bufs=8))
        psum = ctx.enter_context(tc.tile_pool(name="psum", space="PSUM", bufs=2))

        ident = consts.tile([128, 128], BF16)
        make_identity(nc, ident[:])
        eps = consts.tile([128, 1], F32)
        nc.gpsimd.memset(eps, 1e-24)

        def rsqrt_chain(src_tile, scale, tag, sumsq_on_dve=False):
            """[128,1] rscale = 1/sqrt(sumsq(src)*scale) via sumsq + ACT sqrt
            + DVE reciprocal. Returns the [128,1] f32 AP."""
            ssq = stats.tile([128, 1], F32, tag=f"ssq{tag}", name=f"ssq{tag}")
            sq = stats.tile([128, d], F32, tag=f"sq{tag}", name=f"sq{tag}", bufs=3)
            if sumsq_on_dve:
                nc.vector.tensor_mul(sq, src_tile, src_tile)
                nc.vector.reduce_sum(ssq, sq, axis=mybir.AxisListType.X)
                sqrt_scale = scale
            else:
                nc.scalar.activation(
                    out=sq, in_=src_tile, func=mybir.ActivationFunctionType.Square,
                    scale=math.sqrt(scale), accum_out=ssq,
                )
                sqrt_scale = 1.0
            nrm = stats.tile([128, 1], F32, tag=f"nrm{tag}", name=f"nrm{tag}")
            # sqrt(ssq + tiny): tiny avoids 1/0 on zero-padded rows
            nc.scalar.activation(
                out=nrm, in_=ssq, func=mybir.ActivationFunctionType.Sqrt,
                bias=eps[:], scale=sqrt_scale,
            )
            rs = stats.tile([128, 1], F32, tag=f"rs{tag}", name=f"rs{tag}")
            nc.vector.reciprocal(out=rs, in_=nrm)
            return rs

        # ---- phase 1: xnT [k 4x128, n] bf16, with SCALE/||x|| folded in
        xt_ps = [
            psum.tile([128, min(n, 1024)], BF16, tag=f"tp{k % 2}", name=f"xtps{k}")
            for k in range(kt)
        ]
        for b in range(nt):
            x_tile = xpool.tile([128, d], F32, name="x_tile")
            nc.sync.dma_start(out=x_tile, in_=x[b * 128:(b + 1) * 128, :])
            # sumsq of x/SCALE -> rs = SCALE/||x||
            rs = rsqrt_chain(x_tile, 1.0 / (SCALE * SCALE), "x")
            xbf = xpool.tile([128, d], BF16, name="xbf")
            nc.vector.tensor_scalar_mul(xbf, x_tile, rs)
            for k in range(kt):
                nc.tensor.transpose(
                    out=xt_ps[k][:, b * 128:(b + 1) * 128],
                    in_=xbf[:, k * 128:(k + 1) * 128],
                    identity=ident[:],
                )
        xnT = []
        for k in range(kt):
            t = xnt_pool.tile([128, n], BF16, tag=f"xnt{k}", name=f"xnT{k}")
            nc.vector.tensor_copy(t, xt_ps[k])
            xnT.append(t)

        # ---- phase 2: stream W chunks, build wT, matmul, write out
        for g in range(n_chunks):
            c0 = g * 512
            cw = min(512, c_pad - c0)        # chunk class-width (512 or 256)
            tcnt = cw // 128                 # natural 128-row tiles in chunk
            ow = min(512, c_out - c0)        # columns actually written (<=cw)

            wnat = wpool.tile([128, 4, d], F32, name="wnat")
            w_eng = nc.gpsimd if swdge_w else nc.sync
            w_eng.dma_start(
                out=wnat[:, :tcnt, :],
                in_=w[c0:c0 + cw, :].rearrange("(t p) k -> p t k", p=128),
            )

            wt_ps = [
                psum.tile([128, 1024], BF16, tag=f"tp{j}", name=f"wtps{j}")
                for j in range(2)
            ]
            for t in range(tcnt):
                # split sumsq across ACT (t=0) and DVE (t=1..3): ACT is the
                # pacing engine at ~87% busy, DVE has slack
                rsw = rsqrt_chain(wnat[:, t], 1.0, "w", sumsq_on_dve=(t >= 1))
                wbf = wbf_pool.tile([128, d], BF16, name="wbf")
                nc.vector.tensor_scalar_mul(wbf, wnat[:, t], rsw)
                for k in range(kt):
                    j, half = k // 2, k % 2
                    nc.tensor.transpose(
                        out=wt_ps[j][:, half * 512 + t * 128: half * 512 + (t + 1) * 128],
                        in_=wbf[:, k * 128:(k + 1) * 128],
                        identity=ident[:],
                    )
            wt_sb = []
            for j in range(2):
                t2 = wt2_pool.tile([128, 1024], BF16, tag=f"wt2_{j}", name=f"wt2_{j}")
                nc.scalar.copy(t2[:, 0:cw], wt_ps[j][:, 0:cw])
                nc.scalar.copy(t2[:, 512:512 + cw], wt_ps[j][:, 512:512 + cw])
                wt_sb.append(t2)

            for b in range(nt):
                pt = psum.tile([128, 512], F32, tag="opsum", name="pt", bufs=4)
                for k in range(kt):
                    nc.tensor.matmul(
                        pt[:, 0:cw],
                        lhsT=xnT[k][:, b * 128:(b + 1) * 128],
                        rhs=wt_sb[k // 2][:, (k % 2) * 512:(k % 2) * 512 + cw],
                        start=(k == 0), stop=(k == kt - 1),
                    )
                ost = opool.tile([128, 512], F32, name="ost")
                if b % 3 == 2:
                    nc.vector.tensor_copy(ost[:, 0:ow], pt[:, 0:ow])
                else:
                    nc.scalar.copy(ost[:, 0:ow], pt[:, 0:ow])
                nc.sync.dma_start(
                    out=out[b * 128:(b + 1) * 128, c0:c0 + ow],
                    in_=ost[:, 0:ow],
                )
    nc.compile()
    return nc


_NC_CACHE = {}


def _get_nc():
    if "nc" not in _NC_CACHE:
        _NC_CACHE["nc"] = build_nc()
    return _NC_CACHE["nc"]


def prep_in_maps(input, weight):
    x = np.ascontiguousarray(np.asarray(input, dtype=np.float32))
    w = np.asarray(weight, dtype=np.float32).reshape(N_CORES, C_PER, D)
    in_maps = []
    for i in range(N_CORES):
        wp = np.zeros((C_PAD, D), dtype=np.float32)
        wp[:C_PER] = w[i]
        in_maps.append({"x": x, "w": wp})
    return in_maps


def assemble(results, label):
    out = np.empty((N, C_TOTAL), dtype=np.float32)
    for i in range(N_CORES):
        out[:, i * C_PER:(i + 1) * C_PER] = results[i]["out"][:, :C_PER]
    lab = np.asarray(label).astype(np.int64)
    rows = np.arange(N)
    cos_t = out[rows, lab] / np.float32(SCALE)
    sin_t = np.sqrt(np.maximum(1.0 - cos_t * cos_t, 0.0), dtype=np.float32)
    phi = cos_t * np.float32(COS_M) - sin_t * np.float32(SIN_M)
    phi = np.where(cos_t > np.float32(TH), phi, cos_t - np.float32(MM))
    out[rows, lab] = np.float32(SCALE) * phi
    return out


def kernel(input, weight, label):
    nc = _get_nc()
    in_maps = prep_in_maps(input, weight)
    res = run_bass_kernel_spmd(nc, in_maps, list(range(N_CORES)))
    return assemble(res.results, label)



# revision 5
# speedup vs baseline: 1.6154x; 1.6154x over previous
"""ArcFace logits kernel for 8 trn2 NeuronCores (class-axis model parallel).

kernel(input, weight, label) -> [1024, 100000] f32 scaled-margin logits.

Strategy (v2): the O(N*C*D) cosine GEMM runs on device; all O((N+C)*D)
layout/precision prep runs on host so the device program is a pure
matmul + evict + store pipeline near its rooflines:

Host prep (per core shard of 12500 classes):
  - xn = l2norm(x) f64, laid out as xnT [4 kt, 128 d, 1024 n] and cast to
    fp8e4 (TRN e4m3, max 240) -- single rounding, matches ml_dtypes.
  - wn = l2norm(w_shard) f32, laid out wnT [4 kt, 128 d, 12800 c-padded]
    cast fp8e4.  Logical contraction index d = kt*128 + partition.
Device (SPMD, 8 cores):
  - xnT resident in SBUF (4 KB/partition), wnT resident (51 KB/partition,
    loaded in 5 column groups so MMs start after the first group lands).
  - 25 chunks x 8 batch-tiles: 2 DoubleRow fp8 matmuls (contraction 256
    each) accumulate [128,512] f32 in PSUM; evict with x30 scale to fp16
    (ACT/DVE alternating); DMA the [128, <=512] fp16 slab to DRAM out.
Host assemble:
  - concat 8 fp16 slabs -> f32 [1024, 100000]
  - margin positions: cos_t recomputed exactly (f64) from xn/wn rows at
    the 1024 labels, phi overwrites out[rows, label] (same math as ref).

Numerics (validated against the fixed seed-0 data in f64 simulation):
  fp8 x/w + fp16 out -> rel err 0.0165 vs gate 0.02; bf16 mode 0.001.
MODE="bf16" is the fallback (4 plain MMs instead of 2 DoubleRow MMs,
bf16 operands; same structure, ~1.6x slower PE).
"""

import math
from contextlib import ExitStack

import numpy as np
import ml_dtypes

import concourse.bass as bass
import concourse.bacc as bacc
import concourse.mybir as mybir
from concourse.tile import TileContext
from concourse.bass_utils import run_bass_kernel_spmd

F32 = mybir.dt.float32
F16 = mybir.dt.float16
BF16 = mybir.dt.bfloat16
FP8 = mybir.dt.float8e4

N = 1024          # batch
D = 512           # in_features
C_TOTAL = 100000  # out_features
N_CORES = 8
C_PER = C_TOTAL // N_CORES     # 12500 real classes per core
C_PAD = 12800                  # 25 chunks of 512
KT = D // 128                  # 4 k-subtiles
NT = N // 128                  # 8 batch tiles
N_CHUNKS = C_PAD // 512        # 25
GROUPS = 5                     # W streamed in 5 column groups
GW = C_PAD // GROUPS           # 2560 cols per group

SCALE = 30.0
MARGIN = 0.5
COS_M = math.cos(MARGIN)
SIN_M = math.sin(MARGIN)
TH = math.cos(math.pi - MARGIN)
MM = math.sin(math.pi - MARGIN) * MARGIN

MODE = "fp8"   # "fp8" (DoubleRow) | "bf16" (fallback)
PRESCALE = 32.0  # fp8 mode: x,w scaled by 32 before quantization (power of
                 # 2, exact), compensated in the evict scale 30/1024.


def build_nc(mode=MODE):
    in_dt = FP8 if mode == "fp8" else BF16
    ev_scale = SCALE / (PRESCALE * PRESCALE) if mode == "fp8" else SCALE
    nc = bacc.Bacc(None, target_bir_lowering=False, debug=False)
    xt = nc.declare_dram_parameter("xt", [KT, 128, N], in_dt, isOutput=False)
    wt = nc.declare_dram_parameter("wt", [KT, 128, C_PAD], in_dt, isOutput=False)
    out = nc.declare_dram_parameter("out", [N, C_PER], F16, isOutput=True)

    with ExitStack() as ctx:
        tc = ctx.enter_context(TileContext(nc))

        xpool = ctx.enter_context(tc.tile_pool(name="xnt", bufs=1))
        wpool = ctx.enter_context(tc.tile_pool(name="wtp", bufs=1))
        opool = ctx.enter_context(tc.tile_pool(name="opool", bufs=8))
        psum = ctx.enter_context(tc.tile_pool(name="psum", space="PSUM", bufs=2))

        xnt = xpool.tile([128, KT, N], in_dt, name="xnt")
        for k in range(KT):
            nc.sync.dma_start(out=xnt[:, k, :], in_=xt[k])

        wtiles = []
        for gr in range(GROUPS):
            t = wpool.tile([128, KT, GW], in_dt, tag=f"w{gr}", name=f"w{gr}")
            for k in range(KT):
                nc.gpsimd.dma_start(out=t[:, k, :], in_=wt[k, :, gr * GW:(gr + 1) * GW])
            wtiles.append(t)

        chunks_per_group = N_CHUNKS // GROUPS
        for g in range(N_CHUNKS):
            gr = g // chunks_per_group
            l0 = (g % chunks_per_group) * 512
            c0 = g * 512
            ow = min(512, C_PER - c0)          # fp16 cols actually written
            for b in range(NT):
                pt = psum.tile([128, 512], F32, tag="opsum", name="pt", bufs=6)
                if mode == "fp8":
                    for j in range(2):
                        nc.tensor.matmul(
                            pt,
                            lhsT=xnt[:, 2 * j:2 * j + 2, b * 128:(b + 1) * 128],
                            rhs=wtiles[gr][:, 2 * j:2 * j + 2, l0:l0 + 512],
                            start=(j == 0), stop=(j == 1),
                            perf_mode=mybir.MatmulPerfMode.DoubleRow,
                        )
                else:
                    for k in range(KT):
                        nc.tensor.matmul(
                            pt,
                            lhsT=xnt[:, k, b * 128:(b + 1) * 128],
                            rhs=wtiles[gr][:, k, l0:l0 + 512],
                            start=(k == 0), stop=(k == KT - 1),
                        )
                ost = opool.tile([128, 512], F16, name="ost")
                if b % 2 == 0:
                    nc.scalar.activation(
                        out=ost, in_=pt,
                        func=mybir.ActivationFunctionType.Copy, scale=ev_scale,
                    )
                else:
                    nc.vector.tensor_scalar_mul(ost, pt, ev_scale)
                nc.sync.dma_start(
                    out=out[b * 128:(b + 1) * 128, c0:c0 + ow],
                    in_=ost[:, 0:ow],
                )
    nc.compile()
    return nc


_NC_CACHE = {}
_HOST_CTX = {}


def _get_nc():
    if "nc" not in _NC_CACHE:
        _NC_CACHE["nc"] = build_nc()
    return _NC_CACHE["nc"]


def prep_in_maps(input, weight, mode=MODE):
    np_in = ml_dtypes.float8_e4m3 if mode == "fp8" else ml_dtypes.bfloat16
    x64 = np.asarray(input, dtype=np.float64)
    xn = x64 / np.maximum(np.linalg.norm(x64, axis=1, keepdims=True), 1e-12)
    _HOST_CTX["xn"] = xn
    _HOST_CTX["weight"] = weight
    ps = PRESCALE if mode == "fp8" else 1.0
    xt = np.ascontiguousarray(xn.T * ps).reshape(KT, 128, N).astype(np_in)

    w = np.asarray(weight, dtype=np.float32)
    in_maps = []
    for i in range(N_CORES):
        wi = w[i * C_PER:(i + 1) * C_PER]
        nrm = np.sqrt(np.einsum("cd,cd->c", wi, wi, dtype=np.float64))
        wn = wi / np.maximum(nrm, 1e-12).astype(np.float32)[:, None]
        wti = np.zeros((D, C_PAD), dtype=np.float32)
        wti[:, :C_PER] = wn.T * np.float32(ps)
        in_maps.append({"xt": xt, "wt": wti.reshape(KT, 128, C_PAD).astype(np_in)})
    return in_maps


def assemble(results, label):
    out = np.empty((N, C_TOTAL), dtype=np.float32)
    for i in range(N_CORES):
        out[:, i * C_PER:(i + 1) * C_PER] = results[i]["out"].astype(np.float32)
    lab = np.asarray(label).astype(np.int64)
    rows = np.arange(N)
    # exact margin: recompute the 1024 true-class cosines on host in f64
    xn = _HOST_CTX["xn"]
    wrows = np.asarray(_HOST_CTX["weight"], dtype=np.float32)[lab].astype(np.float64)
    wrows /= np.maximum(np.linalg.norm(wrows, axis=1, keepdims=True), 1e-12)
    cos_t = np.einsum("nd,nd->n", xn, wrows)
    sin_t = np.sqrt(np.maximum(1.0 - cos_t * cos_t, 0.0))
    phi = cos_t * COS_M - sin_t * SIN_M
    phi = np.where(cos_t > TH, phi, cos_t - MM)
    out[rows, lab] = (SCALE * phi).astype(np.float32)
    return out


def kernel(input, weight, label):
    nc = _get_nc()
    in_maps = prep_in_maps(input, weight)
    res = run_bass_kernel_spmd(nc, in_maps, list(range(N_CORES)))
    return assemble(res.results, label)


# revision 6
# speedup vs baseline: 2.4261x; 1.5019x over previous
"""ArcFace logits kernel for 8 trn2 NeuronCores (class-axis model parallel).

kernel(input, weight, label) -> [1024, 100000] f32 scaled-margin logits.

Strategy (v2): the O(N*C*D) cosine GEMM runs on device; all O((N+C)*D)
layout/precision prep runs on host so the device program is a pure
matmul + evict + store pipeline near its rooflines:

Host prep (per core shard of 12500 classes):
  - xn = l2norm(x) f64, laid out as xnT [4 kt, 128 d, 1024 n] and cast to
    fp8e4 (TRN e4m3, max 240) -- single rounding, matches ml_dtypes.
  - wn = l2norm(w_shard) f32, laid out wnT [4 kt, 128 d, 12800 c-padded]
    cast fp8e4.  Logical contraction index d = kt*128 + partition.
Device (SPMD, 8 cores):
  - xnT resident in SBUF (4 KB/partition), wnT resident (51 KB/partition,
    loaded in 5 column groups so MMs start after the first group lands).
  - 25 chunks x 8 batch-tiles: 2 DoubleRow fp8 matmuls (contraction 256
    each) accumulate [128,512] f32 in PSUM; evict with x30 scale to fp16
    (ACT/DVE alternating); DMA the [128, <=512] fp16 slab to DRAM out.
Host assemble:
  - concat 8 fp16 slabs -> f32 [1024, 100000]
  - margin positions: cos_t recomputed exactly (f64) from xn/wn rows at
    the 1024 labels, phi overwrites out[rows, label] (same math as ref).

Numerics (validated against the fixed seed-0 data in f64 simulation):
  fp8 x/w + fp16 out -> rel err 0.0165 vs gate 0.02; bf16 mode 0.001.
MODE="bf16" is the fallback (4 plain MMs instead of 2 DoubleRow MMs,
bf16 operands; same structure, ~1.6x slower PE).
"""

import math
from contextlib import ExitStack

import numpy as np
import ml_dtypes

import concourse.bass as bass
import concourse.bacc as bacc
import concourse.mybir as mybir
from concourse.tile import TileContext
from concourse.bass_utils import run_bass_kernel_spmd

F32 = mybir.dt.float32
F16 = mybir.dt.float16
BF16 = mybir.dt.bfloat16
FP8 = mybir.dt.float8e4

N = 1024          # batch
D = 512           # in_features
C_TOTAL = 100000  # out_features
N_CORES = 8
C_PER = C_TOTAL // N_CORES     # 12500 real classes per core
C_PAD = 12800                  # 25 chunks of 512
KT = D // 128                  # 4 k-subtiles
NT = N // 128                  # 8 batch tiles
N_CHUNKS = C_PAD // 512        # 25
GROUPS = 5                     # W streamed in 5 column groups
GW = C_PAD // GROUPS           # 2560 cols per group

SCALE = 30.0
MARGIN = 0.5
COS_M = math.cos(MARGIN)
SIN_M = math.sin(MARGIN)
TH = math.cos(math.pi - MARGIN)
MM = math.sin(math.pi - MARGIN) * MARGIN

MODE = "fp8"   # "fp8" (DoubleRow) | "bf16" (fallback)
PRESCALE = 32.0  # fp8 mode: x,w scaled by 32 before quantization (power of
                 # 2, exact), compensated in the evict scale 30/1024.


def build_nc(mode=MODE):
    in_dt = FP8 if mode == "fp8" else BF16
    ev_scale = SCALE / (PRESCALE * PRESCALE) if mode == "fp8" else SCALE
    nc = bacc.Bacc(None, target_bir_lowering=False, debug=False)
    xt = nc.declare_dram_parameter("xt", [KT, 128, N], in_dt, isOutput=False)
    wt = nc.declare_dram_parameter("wt", [KT, 128, C_PAD], in_dt, isOutput=False)
    out = nc.declare_dram_parameter("out", [N, C_PER], F16, isOutput=True)

    # quads: groups of up to 4 chunks -> one wide out-write per (quad, b).
    quads = [(q * 4, min(4, N_CHUNKS - q * 4)) for q in range((N_CHUNKS + 3) // 4)]

    with ExitStack() as ctx:
        tc = ctx.enter_context(TileContext(nc))

        xpool = ctx.enter_context(tc.tile_pool(name="xnt", bufs=1))
        wpool = ctx.enter_context(tc.tile_pool(name="wtp", bufs=1))
        opool = ctx.enter_context(tc.tile_pool(name="opool", bufs=6))
        psum = ctx.enter_context(tc.tile_pool(name="psum", space="PSUM", bufs=2))

        xnt = xpool.tile([128, KT, N], in_dt, name="xnt")
        for k in range(KT):
            nc.sync.dma_start(out=xnt[:, k, :], in_=xt[k])

        wtiles = []
        for qi, (g0, qw) in enumerate(quads):
            t = wpool.tile([128, KT, qw * 512], in_dt, tag=f"w{qi}", name=f"w{qi}")
            for k in range(KT):
                nc.gpsimd.dma_start(
                    out=t[:, k, :], in_=wt[k, :, g0 * 512:(g0 + qw) * 512]
                )
            wtiles.append(t)

        for qi, (g0, qw) in enumerate(quads):
            for b in range(NT):
                bs = slice(b * 128, (b + 1) * 128)
                pts = [
                    psum.tile([128, 512], F32, tag="opsum", name=f"pt{gi}", bufs=8)
                    for gi in range(qw)
                ]
                if mode == "fp8":
                    # j outer so consecutive MMs share the stationary lhsT
                    for j in range(2):
                        for gi in range(qw):
                            nc.tensor.matmul(
                                pts[gi],
                                lhsT=xnt[:, 2 * j:2 * j + 2, bs],
                                rhs=wtiles[qi][:, 2 * j:2 * j + 2, gi * 512:(gi + 1) * 512],
                                start=(j == 0), stop=(j == 1),
                                perf_mode=mybir.MatmulPerfMode.DoubleRow,
                            )
                else:
                    for k in range(KT):
                        for gi in range(qw):
                            nc.tensor.matmul(
                                pts[gi],
                                lhsT=xnt[:, k, bs],
                                rhs=wtiles[qi][:, k, gi * 512:(gi + 1) * 512],
                                start=(k == 0), stop=(k == KT - 1),
                            )
                ost = opool.tile([128, qw * 512], F16, name="ost", tag=f"ost{qw}")
                for gi in range(qw):
                    osl = ost[:, gi * 512:(gi + 1) * 512]
                    if (b + gi) % 2 == 0:
                        nc.scalar.activation(
                            out=osl, in_=pts[gi],
                            func=mybir.ActivationFunctionType.Copy, scale=ev_scale,
                        )
                    else:
                        nc.vector.tensor_scalar_mul(osl, pts[gi], ev_scale)
                ow = min(qw * 512, C_PER - g0 * 512)   # fp16 cols actually written
                nc.sync.dma_start(
                    out=out[bs, g0 * 512:g0 * 512 + ow],
                    in_=ost[:, 0:ow],
                )
    nc.compile()
    return nc


_NC_CACHE = {}
_HOST_CTX = {}


def _get_nc():
    if "nc" not in _NC_CACHE:
        _NC_CACHE["nc"] = build_nc()
    return _NC_CACHE["nc"]


def prep_in_maps(input, weight, mode=MODE):
    np_in = ml_dtypes.float8_e4m3 if mode == "fp8" else ml_dtypes.bfloat16
    x64 = np.asarray(input, dtype=np.float64)
    xn = x64 / np.maximum(np.linalg.norm(x64, axis=1, keepdims=True), 1e-12)
    _HOST_CTX["xn"] = xn
    _HOST_CTX["weight"] = weight
    ps = PRESCALE if mode == "fp8" else 1.0
    xt = np.ascontiguousarray(xn.T * ps).reshape(KT, 128, N).astype(np_in)

    w = np.asarray(weight, dtype=np.float32)
    in_maps = []
    for i in range(N_CORES):
        wi = w[i * C_PER:(i + 1) * C_PER]
        nrm = np.sqrt(np.einsum("cd,cd->c", wi, wi, dtype=np.float64))
        wn = wi / np.maximum(nrm, 1e-12).astype(np.float32)[:, None]
        wti = np.zeros((D, C_PAD), dtype=np.float32)
        wti[:, :C_PER] = wn.T * np.float32(ps)
        in_maps.append({"xt": xt, "wt": wti.reshape(KT, 128, C_PAD).astype(np_in)})
    return in_maps


def assemble(results, label):
    out = np.empty((N, C_TOTAL), dtype=np.float32)
    for i in range(N_CORES):
        out[:, i * C_PER:(i + 1) * C_PER] = results[i]["out"].astype(np.float32)
    lab = np.asarray(label).astype(np.int64)
    rows = np.arange(N)
    # exact margin: recompute the 1024 true-class cosines on host in f64
    xn = _HOST_CTX["xn"]
    wrows = np.asarray(_HOST_CTX["weight"], dtype=np.float32)[lab].astype(np.float64)
    wrows /= np.maximum(np.linalg.norm(wrows, axis=1, keepdims=True), 1e-12)
    cos_t = np.einsum("nd,nd->n", xn, wrows)
    sin_t = np.sqrt(np.maximum(1.0 - cos_t * cos_t, 0.0))
    phi = cos_t * COS_M - sin_t * SIN_M
    phi = np.where(cos_t > TH, phi, cos_t - MM)
    out[rows, lab] = (SCALE * phi).astype(np.float32)
    return out


def kernel(input, weight, label):
    nc = _get_nc()
    in_maps = prep_in_maps(input, weight)
    res = run_bass_kernel_spmd(nc, in_maps, list(range(N_CORES)))
    return assemble(res.results, label)
